# revision 1
# baseline (speedup 1.0000x reference)
"""Trainium2 Bass kernel for causal multi-head attention (B=2, T=4096, C=768, H=12).

Sharding: 8 cores = 2 (batch) x 4 (head groups of 3). Each core computes, for its
batch element b and its 3 heads:
  - Q^T/K^T projection (transposed layout, fused q/k bias)
  - V projection (natural layout, ones column appended for free softmax sums)
  - causal flash attention in S^T = [keys, queries] orientation
  - row-sharded output projection -> partial [T, C] output
Host sums the 4 partial outputs per batch element and adds the bias terms.

All matmuls run as float32r (FP22, full PE speed with N>=256), fp32 accumulate.

Layout note: the PE requires lhsT and rhs at the same SBUF base partition, so
Q^T/K^T are packed as: free block 0 = Q(h0)@p0-63 | Q(h1)@p64-127, block 1 =
K(h0)|K(h1) likewise, blocks 2/3 = Q(h2)/K(h2) @ p0-63.
"""

import os
import sys

for _p in ("/opt/trn_rl_repo", "/root/.axon_site/_ro/trn_rl_repo"):
    if os.path.isdir(_p) and _p not in sys.path:
        sys.path.insert(0, _p)

import ml_dtypes
import numpy as np

import concourse.bass as bass
import concourse.mybir as mybir
import concourse.tile as tile
from concourse import bacc, bass_utils

# Problem constants (hardcoded per harness contract)
B, T_FULL, C = 2, 4096, 768
H, D = 12, 64
N_CORES = 8
HPC = 3          # heads per core
GROUPS = 4       # head groups (cores per batch element)

F32 = mybir.dt.float32
F32R = mybir.dt.float32r
BF16 = mybir.dt.bfloat16

# dtype config: which tensors run reduced precision.
# keys: xt (x^T + qk/v proj inputs), qkt (Q^T/K^T), e (probabilities + masks),
#       v1 (V|ones), yt (normalized attn out), wqk, wv, wout
CFG_F32R = {k: F32R for k in ("xt", "qkt", "e", "v1", "yt", "wqk", "wv", "wout")}
CFG_BF16 = {k: BF16 for k in CFG_F32R}
CFG_MIXED = dict(CFG_F32R, qkt=BF16, e=BF16)
CFGS = {"f32r": CFG_F32R, "bf16": CFG_BF16, "mixed": CFG_MIXED}  # mixed: do not use (32/16-bit matmul mix fails walrus)


def _np_dt(dt):
    return ml_dtypes.bfloat16 if dt == BF16 else np.float32


def build_nc(T=T_FULL, cfg=CFG_F32R):
    """Build the per-core Bass module. T must be a multiple of 512."""
    QB = 512                 # query block
    KC = 128                 # key chunk
    NTB = T // QB            # token blocks
    NCC = C // 128           # contraction chunks (6)
    NKC = T // KC            # key chunks total
    CS = HPC * (D + 1)       # V|ones chunk stride (195)
    CO = C                   # output channels
    VW = HPC * D             # v width (192)
    VWP = 256                # padded v proj width

    nc = bacc.Bacc(None, target_bir_lowering=False, debug=False)

    xT_d = nc.dram_tensor("xT", [C, T], cfg["xt"], kind="ExternalInput")
    wqk_d = nc.dram_tensor("wqk", [C, 4 * 128], cfg["wqk"], kind="ExternalInput")
    bqk_d = nc.dram_tensor("bqk", [128, 4], F32, kind="ExternalInput")
    wv_d = nc.dram_tensor("wv", [C, VWP], cfg["wv"], kind="ExternalInput")
    wout_d = nc.dram_tensor("wout", [VW, CO], cfg["wout"], kind="ExternalInput")
    masks_d = nc.dram_tensor("masks", [128, 4 * QB], cfg["e"], kind="ExternalInput")
    ident_d = nc.dram_tensor("ident", [128, 64], cfg["qkt"], kind="ExternalInput")
    out_d = nc.dram_tensor("out", [T, CO], F32, kind="ExternalOutput")

    # per-head addressing into the packed qkt tile
    def qbase(h):
        return 0 if h in (0, 2) else 64

    def qoff(h):
        return 0 if h < 2 else 2 * T

    def koff(h):
        return T if h < 2 else 3 * T

    with tile.TileContext(nc) as tc:
        with (
            tc.tile_pool(name="singles", bufs=1) as singles,
            tc.tile_pool(name="xt", bufs=12) as xt_pool,
            tc.tile_pool(name="e", bufs=4) as e_pool,
            tc.tile_pool(name="yt", bufs=6) as yt_pool,
            tc.tile_pool(name="nrm", bufs=3) as nrm_pool,
            tc.tile_pool(name="ostage", bufs=3) as out_pool,
            tc.tile_pool(name="ps", bufs=2, space="PSUM") as psum_s,
            tc.tile_pool(name="pz", bufs=2, space="PSUM") as psum_z,
            tc.tile_pool(name="po", bufs=2, space="PSUM") as psum_o,
        ):
            # Persistent SBUF tensors
            qkt = singles.tile([128, 4 * T], cfg["qkt"])
            v1 = singles.tile([128, NKC * CS], cfg["v1"])      # V|ones, keys on partitions
            wqk_s = singles.tile([128, NCC * 512], cfg["wqk"])
            wv_s = singles.tile([128, NCC * VWP], cfg["wv"])
            wout_s = singles.tile([64, HPC * CO], cfg["wout"])
            masks_s = singles.tile([128, 4 * QB], cfg["e"])
            bqk_s = singles.tile([128, 4], F32)
            ident_s = singles.tile([128, 64], cfg["qkt"])
            ktmp = singles.tile([128, 512], cfg["qkt"])

            # ones columns of v1 (memset can't write f32r; DVE copy rounds)
            ones_c = singles.tile([128, 1], F32)
            nc.vector.memset(ones_c[:], 1.0)
            ones_dst = v1[:].rearrange("p (k h x) -> p k h x", h=HPC, x=D + 1)[
                :, :, :, D:D + 1
            ]
            nc.vector.tensor_copy(ones_dst, ones_c.to_broadcast([128, NKC, HPC, 1]))
            nc.sync.dma_start(out=bqk_s[:], in_=bqk_d.ap())
            nc.sync.dma_start(out=ident_s[:], in_=ident_d.ap())

            for tb in range(NTB):
                # ---- load x^T tiles for this token block ----
                xt = []
                for c in range(NCC):
                    t_ = xt_pool.tile([128, QB], cfg["xt"], tag="xt")
                    nc.sync.dma_start(
                        out=t_[:],
                        in_=xT_d.ap()[c * 128:(c + 1) * 128, tb * QB:(tb + 1) * QB],
                    )
                    xt.append(t_)
                    if tb == 0:
                        nc.sync.dma_start(
                            out=wqk_s[:, c * 512:(c + 1) * 512],
                            in_=wqk_d.ap()[c * 128:(c + 1) * 128, :],
                        )
                if tb == 0:
                    for c in range(NCC):
                        nc.sync.dma_start(
                            out=wv_s[:, c * VWP:(c + 1) * VWP],
                            in_=wv_d.ap()[c * 128:(c + 1) * 128, :],
                        )
                    nc.sync.dma_start(out=masks_s[:], in_=masks_d.ap())
                    for h_ in range(HPC):
                        nc.sync.dma_start(
                            out=wout_s[:, h_ * CO:(h_ + 1) * CO],
                            in_=wout_d.ap()[h_ * D:(h_ + 1) * D, :],
                        )

                # ---- Q^T / K^T projection ----
                # 3 M-tiles: 0 = Q(h0)|Q(h1), 1 = K(h0)|K(h1), 2 = Q(h2)|K(h2).
                # K(h2) lands on psum partitions 64-127 but must live at 0-63
                # (same base as Q(h2) for the S matmul): identity-shift via PE.
                for mt in range(3):
                    ps = psum_s.tile([128, 2 * QB], F32, tag="ps")
                    for c in range(NCC):
                        nc.tensor.matmul(
                            ps[:, 0:QB],
                            lhsT=(wqk_s[:, c * 512 + mt * 128: c * 512 + (mt + 1) * 128]),
                            rhs=(xt[c][:]),
                            start=(c == 0),
                            stop=(c == NCC - 1),
                        )
                    if mt < 2:
                        dst_off = mt * T + tb * QB
                        nc.vector.tensor_scalar(
                            out=qkt[:, dst_off: dst_off + QB],
                            in0=ps[:, 0:QB],
                            scalar1=bqk_s[:, mt:mt + 1],
                            scalar2=None,
                            op0=mybir.AluOpType.add,
                        )
                    else:
                        # q2 -> block 2 directly
                        nc.vector.tensor_scalar(
                            out=qkt[0:64, 2 * T + tb * QB: 2 * T + (tb + 1) * QB],
                            in0=ps[0:64, 0:QB],
                            scalar1=bqk_s[0:64, 2:3],
                            scalar2=None,
                            op0=mybir.AluOpType.add,
                        )
                        # k2: psum[64:128] -> sbuf (bias fused) -> PE shift to base 0
                        nc.vector.tensor_scalar(
                            out=ktmp[64:128, 0:QB],
                            in0=ps[64:128, 0:QB],
                            scalar1=bqk_s[64:128, 2:3],
                            scalar2=None,
                            op0=mybir.AluOpType.add,
                        )
                        ps2 = psum_s.tile([128, 2 * QB], F32, tag="ps")
                        nc.tensor.matmul(
                            ps2[0:64, 0:QB],
                            lhsT=(ident_s[64:128, :]),
                            rhs=(ktmp[64:128, 0:QB]),
                            start=True,
                            stop=True,
                        )
                        nc.vector.tensor_copy(
                            qkt[0:64, 3 * T + tb * QB: 3 * T + (tb + 1) * QB],
                            ps2[0:64, 0:QB],
                        )

                # ---- V projection (natural layout + ones) ----
                for ts in range(QB // 128):
                    pv = psum_z.tile([128, 512], F32, tag="pz")
                    for c in range(NCC):
                        nc.tensor.matmul(
                            pv[:, 0:VWP],
                            lhsT=(xt[c][:, ts * 128:(ts + 1) * 128]),
                            rhs=(wv_s[:, c * VWP:(c + 1) * VWP]),
                            start=(c == 0),
                            stop=(c == NCC - 1),
                        )
                    kc = tb * (QB // 128) + ts
                    dst = v1[:, kc * CS:(kc + 1) * CS].rearrange(
                        "p (h x) -> p h x", x=D + 1
                    )[:, :, 0:D]
                    src = pv[:, 0:VW].rearrange("p (h d) -> p h d", d=D)
                    nc.vector.tensor_copy(dst, src)

                # ---- causal attention for query block j = tb ----
                j = tb
                yts = []
                for h in range(HPC):
                    qb_, qo_, ko_ = qbase(h), qoff(h), koff(h)
                    pz = psum_z.tile([128, 512], F32, tag="pz")
                    nchunks = 4 * (j + 1)
                    nbatches = nchunks // 2
                    for m in range(nbatches):
                        ps = psum_s.tile([128, 2 * QB], F32, tag="ps")
                        for u in range(2):
                            n = 2 * m + u
                            trim = max(0, (n - 4 * j) * KC)
                            nc.tensor.matmul(
                                ps[:, u * QB + trim:(u + 1) * QB],
                                lhsT=(qkt[qb_:qb_ + 64, ko_ + n * KC: ko_ + (n + 1) * KC]),
                                rhs=(qkt[qb_:qb_ + 64, qo_ + j * QB + trim: qo_ + (j + 1) * QB]),
                                start=True,
                                stop=True,
                            )
                        e = e_pool.tile([128, 2 * QB], cfg["e"], tag="e")
                        trims = [max(0, (2 * m + u - 4 * j) * KC) for u in range(2)]
                        if trims[0] == 0 and trims[1] == 0:
                            nc.scalar.activation(
                                e[:], ps[:], mybir.ActivationFunctionType.Exp,
                                scale=0.125,
                            )
                        else:
                            for u in range(2):
                                lo = u * QB + trims[u]
                                hi = (u + 1) * QB
                                nc.scalar.activation(
                                    e[:, lo:hi], ps[:, lo:hi],
                                    mybir.ActivationFunctionType.Exp,
                                    scale=0.125,
                                )
                        for u in range(2):
                            n = 2 * m + u
                            cdiag = n - 4 * j
                            if cdiag >= 0:
                                trim = cdiag * KC
                                nc.vector.tensor_mul(
                                    e[:, u * QB + trim:(u + 1) * QB],
                                    e[:, u * QB + trim:(u + 1) * QB],
                                    masks_s[:, cdiag * QB + trim:(cdiag + 1) * QB],
                                )
                        for u in range(2):
                            n = 2 * m + u
                            trim = max(0, (n - 4 * j) * KC)
                            nc.tensor.matmul(
                                pz[0:D + 1, trim:QB],
                                lhsT=(v1[:, n * CS + h * (D + 1): n * CS + (h + 1) * (D + 1)]),
                                rhs=(e[:, u * QB + trim:(u + 1) * QB]),
                                start=(m == 0 and u == 0),
                                stop=(m == nbatches - 1 and u == 1),
                            )
                    # normalize: y = z * (1/sums), broadcast along partitions
                    # 1/sums as exp(-ln(sums)) on ScalarE: ~2ULP, keeps the
                    # slow DVE reciprocal off the psum-recycle critical path
                    # (ln+exp share one ACT table set).
                    lns = nrm_pool.tile([1, QB], F32, tag="lns")
                    nc.scalar.activation(
                        lns[:], pz[D:D + 1, 0:QB], mybir.ActivationFunctionType.Ln
                    )
                    rc = nrm_pool.tile([1, QB], F32, tag="rc")
                    nc.scalar.activation(
                        rc[:], lns[:], mybir.ActivationFunctionType.Exp, scale=-1.0
                    )
                    bc = nrm_pool.tile([64, QB], F32, tag="bc")
                    nc.gpsimd.partition_broadcast(bc[:], rc[:])
                    yt = yt_pool.tile([64, QB], cfg["yt"], tag="yt")
                    nc.vector.tensor_mul(yt[:], pz[0:D, 0:QB], bc[:])
                    yts.append(yt)

                # ---- output projection for this query block ----
                for ts in range(QB // 128):
                    ot = out_pool.tile([128, CO], F32, tag="ot")
                    for half in range(2):
                        po = psum_o.tile([128, 384], F32, tag="po")
                        for h in range(HPC):
                            nc.tensor.matmul(
                                po[:],
                                lhsT=(yts[h][:, ts * 128:(ts + 1) * 128]),
                                rhs=(wout_s[:, h * CO + half * 384: h * CO + (half + 1) * 384]),
                                start=(h == 0),
                                stop=(h == HPC - 1),
                            )
                        nc.vector.tensor_copy(ot[:, half * 384:(half + 1) * 384], po[:])
                    row = j * QB + ts * 128
                    nc.sync.dma_start(out=out_d.ap()[row:row + 128, :], in_=ot[:])

    nc.compile()
    return nc


def make_masks():
    """Diagonal-block masks: masks[k, c*512 + q] = 1.0 iff 128*c + k <= q."""
    QB = 512
    m = np.zeros((128, 4 * QB), dtype=np.float32)
    for c in range(4):
        k = np.arange(128)[:, None]
        q = np.arange(QB)[None, :]
        m[:, c * QB:(c + 1) * QB] = (128 * c + k <= q).astype(np.float32)
    return m


def make_core_inputs(x, Wqkv, bqkv, core, T=T_FULL, cfg=CFG_F32R):
    """Host-side shard prep for one core."""
    b, g = divmod(core, GROUPS)
    h0 = HPC * g  # first global head of this core
    xT = np.ascontiguousarray(x[b].T).astype(np.float32)          # [C, T]

    def wq(h):
        return Wqkv[:, h * D:(h + 1) * D]

    def wk(h):
        return Wqkv[:, C + h * D: C + (h + 1) * D]

    def bq(h):
        return bqkv[h * D:(h + 1) * D]

    def bk(h):
        return bqkv[C + h * D: C + (h + 1) * D]

    z64 = np.zeros((C, D), dtype=np.float32)
    wqk = np.concatenate(
        [wq(h0), wq(h0 + 1), wk(h0), wk(h0 + 1), wq(h0 + 2), wk(h0 + 2), z64, z64],
        axis=1,
    ).astype(np.float32)                                           # [C, 512]
    zb = np.zeros(D, dtype=np.float32)
    bqk = np.stack(
        [
            np.concatenate([bq(h0), bq(h0 + 1)]),
            np.concatenate([bk(h0), bk(h0 + 1)]),
            np.concatenate([bq(h0 + 2), bk(h0 + 2)]),
            np.concatenate([zb, zb]),
        ],
        axis=1,
    ).astype(np.float32)                                           # [128, 4]
    wv = np.zeros((C, 256), dtype=np.float32)
    wv[:, : HPC * D] = Wqkv[:, 2 * C + g * HPC * D: 2 * C + (g + 1) * HPC * D]
    return {
        "xT": np.ascontiguousarray(xT).astype(_np_dt(cfg["xt"])),
        "wqk": np.ascontiguousarray(wqk).astype(_np_dt(cfg["wqk"])),
        "bqk": np.ascontiguousarray(bqk),
        "wv": wv.astype(_np_dt(cfg["wv"])),
        "masks": make_masks().astype(_np_dt(cfg["e"])),
        "ident": np.concatenate(
            [np.zeros((64, 64), np.float32), np.eye(64, dtype=np.float32)]
        ).astype(_np_dt(cfg["qkt"])),
    }


_NC_CACHE = {}


def kernel(x, Wqkv, bqkv, Wout, bout):
    x = np.asarray(x, dtype=np.float32)
    Wqkv = np.asarray(Wqkv, dtype=np.float32)
    bqkv = np.asarray(bqkv, dtype=np.float32)
    Wout = np.asarray(Wout, dtype=np.float32)
    bout = np.asarray(bout, dtype=np.float32)
    T = x.shape[1]

    cfg_name = os.environ.get("KERNEL_CFG", "f32r")
    cfg = CFGS[cfg_name]
    key = (T, cfg_name)
    if key not in _NC_CACHE:
        _NC_CACHE[key] = build_nc(T, cfg)
    nc = _NC_CACHE[key]

    in_maps = []
    for core in range(N_CORES):
        b, g = divmod(core, GROUPS)
        m = make_core_inputs(x, Wqkv, bqkv, core, T, cfg)
        m["wout"] = np.ascontiguousarray(
            Wout[g * HPC * D:(g + 1) * HPC * D, :]
        ).astype(_np_dt(cfg["wout"]))
        in_maps.append(m)

    trace = bool(int(os.environ.get("KERNEL_TRACE", "0")))
    res = bass_utils.run_bass_kernel_spmd(
        nc, in_maps, core_ids=list(range(N_CORES)), trace=trace,
    )
    if trace and res.exec_time_ns is not None:
        print(f"HW exec time: {res.exec_time_ns} ns")
        if res.instructions_and_trace is not None:
            print(f"trace: {res.instructions_and_trace[1]}")

    out = np.zeros((B, T, C), dtype=np.float32)
    for b in range(B):
        for g in range(GROUPS):
            out[b] += res.results[b * GROUPS + g]["out"]
    # host bias compensation: v-bias flows through Wout as a constant row; + bout
    extra = bqkv[2 * C: 3 * C] @ Wout + bout
    out += extra[None, None, :]
    return out



# revision 4
# speedup vs baseline: 1.1207x; 1.1207x over previous
"""Trainium2 Bass kernel for causal multi-head attention (B=2, T=4096, C=768, H=12).

Sharding: 8 cores = 2 (batch) x 4 (head groups of 3). Each core computes, for its
batch element b and its 3 heads:
  - Q^T/K^T projection (transposed layout, fused q/k bias)
  - V projection (natural layout, ones column appended for free softmax sums)
  - causal flash attention in S^T = [keys, queries] orientation
  - row-sharded output projection -> partial [T, C] output
Host sums the 4 partial outputs per batch element and adds the bias terms.

All matmuls run as float32r (FP22: 1 row/cycle at N>=256), fp32 accumulate.

v2 structure (from trace analysis of the v1 baseline):
  - Q^T and K^T live in SEPARATE SBUF tiles (same-tile operands serialize the PE).
  - The flash inner loop is software-pipelined: the S matmuls of batch m+1 are
    emitted before exp/mask/AV of batch m, so the PE never waits on the ACT
    engine; batches are flattened across the 3 heads of a block.
  - Softmax denominators use a DVE reciprocal (reciprocal_approx_fast) instead
    of Ln/Exp on ACT: the Ln table load thrashed the ACT tables 48x per core.

Layout note: the PE requires lhsT and rhs at the same SBUF base partition, so
Q^T/K^T are packed as: free block 0 = h0@p0-63 | h1@p64-127, block 1 = h2@p0-63.
"""

import os
import sys

for _p in ("/opt/trn_rl_repo", "/root/.axon_site/_ro/trn_rl_repo"):
    if os.path.isdir(_p) and _p not in sys.path:
        sys.path.insert(0, _p)

import ml_dtypes
import numpy as np

import concourse.bass as bass
import concourse.mybir as mybir
import concourse.tile as tile
from concourse import bacc, bass_utils

# Problem constants (hardcoded per harness contract)
B, T_FULL, C = 2, 4096, 768
H, D = 12, 64
N_CORES = 8
HPC = 3          # heads per core
GROUPS = 4       # head groups (cores per batch element)

F32 = mybir.dt.float32
F32R = mybir.dt.float32r


def _np_dt(dt):
    return ml_dtypes.bfloat16 if dt == mybir.dt.bfloat16 else np.float32


def build_nc(T=T_FULL):
    """Build the per-core Bass module. T must be a multiple of 512."""
    QB = 512                 # query block
    KC = 128                 # key chunk
    NTB = T // QB            # token blocks
    NCC = C // 128           # contraction chunks (6)
    NKC = T // KC            # key chunks total
    CS = HPC * (D + 1)       # V|ones chunk stride (195)
    CO = C                   # output channels
    VW = HPC * D             # v width (192)
    VWP = 256                # padded v proj width

    nc = bacc.Bacc(None, target_bir_lowering=False, debug=False)

    xT_d = nc.dram_tensor("xT", [C, T], F32R, kind="ExternalInput")
    wqk_d = nc.dram_tensor("wqk", [C, 4 * 128], F32R, kind="ExternalInput")
    bqk_d = nc.dram_tensor("bqk", [128, 4], F32, kind="ExternalInput")
    wv_d = nc.dram_tensor("wv", [C, VWP], F32R, kind="ExternalInput")
    wout_d = nc.dram_tensor("wout", [VW, CO], F32R, kind="ExternalInput")
    masks_d = nc.dram_tensor("masks", [128, 4 * QB], F32R, kind="ExternalInput")
    ident_d = nc.dram_tensor("ident", [128, 64], F32R, kind="ExternalInput")
    out_d = nc.dram_tensor("out", [T, CO], F32, kind="ExternalOutput")

    # per-head addressing into qt/kt tiles: block 0 = h0@p0-63|h1@p64-127,
    # block 1 = h2@p0-63
    def qbase(h):
        return 64 if h == 1 else 0

    def hoff(h):
        return T if h == 2 else 0

    with tile.TileContext(nc) as tc:
        with (
            tc.tile_pool(name="singles", bufs=1) as singles,
            tc.tile_pool(name="xt", bufs=12) as xt_pool,
            tc.tile_pool(name="e", bufs=4) as e_pool,
            tc.tile_pool(name="yt", bufs=6) as yt_pool,
            tc.tile_pool(name="nrm", bufs=3) as nrm_pool,
            tc.tile_pool(name="ostage", bufs=3) as out_pool,
            tc.tile_pool(name="ps", bufs=2, space="PSUM") as psum_s,
            tc.tile_pool(name="pz", bufs=2, space="PSUM") as psum_z,
            tc.tile_pool(name="po", bufs=2, space="PSUM") as psum_o,
        ):
            # Persistent SBUF tensors
            qt = singles.tile([128, 2 * T], F32R)
            kt = singles.tile([128, 2 * T], F32R)
            v1 = singles.tile([128, NKC * CS], F32R)      # V|ones, keys on partitions
            wqk_s = singles.tile([128, NCC * 512], F32R)
            wv_s = singles.tile([128, NCC * VWP], F32R)
            wout_s = singles.tile([64, HPC * CO], F32R)
            masks_s = singles.tile([128, 4 * QB], F32R)
            bqk_s = singles.tile([128, 4], F32)
            ident_s = singles.tile([128, 64], F32R)
            ktmp = singles.tile([128, 512], F32R)

            # ones columns of v1 (memset can't write f32r; DVE copy rounds)
            ones_c = singles.tile([128, 1], F32)
            nc.vector.memset(ones_c[:], 1.0)
            ones_dst = v1[:].rearrange("p (k h x) -> p k h x", h=HPC, x=D + 1)[
                :, :, :, D:D + 1
            ]
            nc.vector.tensor_copy(ones_dst, ones_c.to_broadcast([128, NKC, HPC, 1]))
            nc.sync.dma_start(out=bqk_s[:], in_=bqk_d.ap())
            nc.sync.dma_start(out=ident_s[:], in_=ident_d.ap())

            def emit_eav(j, h, m, nb, ps, pz, yts):
                """exp -> mask -> AV for batch (h, m); normalize at last batch."""
                e = e_pool.tile([128, 2 * QB], F32R, tag="e")
                trims = [max(0, (2 * m + u - 4 * j) * KC) for u in range(2)]
                if trims[0] == 0 and trims[1] == 0:
                    nc.scalar.activation(
                        e[:], ps[:], mybir.ActivationFunctionType.Exp,
                        scale=0.125,
                    )
                else:
                    for u in range(2):
                        lo = u * QB + trims[u]
                        hi = (u + 1) * QB
                        nc.scalar.activation(
                            e[:, lo:hi], ps[:, lo:hi],
                            mybir.ActivationFunctionType.Exp,
                            scale=0.125,
                        )
                for u in range(2):
                    n = 2 * m + u
                    cdiag = n - 4 * j
                    if cdiag >= 0:
                        trim = cdiag * KC
                        nc.vector.tensor_mul(
                            e[:, u * QB + trim:(u + 1) * QB],
                            e[:, u * QB + trim:(u + 1) * QB],
                            masks_s[:, cdiag * QB + trim:(cdiag + 1) * QB],
                        )
                for u in range(2):
                    n = 2 * m + u
                    trim = max(0, (n - 4 * j) * KC)
                    nc.tensor.matmul(
                        pz[0:D + 1, trim:QB],
                        lhsT=(v1[:, n * CS + h * (D + 1): n * CS + (h + 1) * (D + 1)]),
                        rhs=(e[:, u * QB + trim:(u + 1) * QB]),
                        start=(m == 0 and u == 0),
                        stop=(m == nb - 1 and u == 1),
                    )
                if m == nb - 1:
                    # normalize: y = z * (1/sums). ACT Copy stages the sums row
                    # from psum partition 64 to sbuf partition 0 (Copy is in
                    # every ACT table set -> no table reload, unlike the v1
                    # Ln/Exp trick which thrashed tables 48x). Reciprocal runs
                    # on DVE; broadcast along partitions on GPSIMD.
                    sums = nrm_pool.tile([1, QB], F32, tag="sums")
                    nc.scalar.activation(
                        sums[:], pz[D:D + 1, 0:QB],
                        mybir.ActivationFunctionType.Copy,
                    )
                    rc = nrm_pool.tile([1, QB], F32, tag="rc")
                    nc.vector.reciprocal_approx_fast(out=rc[:], in_=sums[:])
                    bc = nrm_pool.tile([64, QB], F32, tag="bc")
                    nc.gpsimd.partition_broadcast(bc[:], rc[:])
                    yt = yt_pool.tile([64, QB], F32R, tag="yt")
                    nc.vector.tensor_mul(yt[:], pz[0:D, 0:QB], bc[:])
                    yts.append(yt)

            for tb in range(NTB):
                # ---- load x^T tiles for this token block ----
                xt = []
                for c in range(NCC):
                    t_ = xt_pool.tile([128, QB], F32R, tag="xt")
                    nc.sync.dma_start(
                        out=t_[:],
                        in_=xT_d.ap()[c * 128:(c + 1) * 128, tb * QB:(tb + 1) * QB],
                    )
                    xt.append(t_)
                    if tb == 0:
                        nc.sync.dma_start(
                            out=wqk_s[:, c * 512:(c + 1) * 512],
                            in_=wqk_d.ap()[c * 128:(c + 1) * 128, :],
                        )
                if tb == 0:
                    for c in range(NCC):
                        nc.sync.dma_start(
                            out=wv_s[:, c * VWP:(c + 1) * VWP],
                            in_=wv_d.ap()[c * 128:(c + 1) * 128, :],
                        )
                    nc.sync.dma_start(out=masks_s[:], in_=masks_d.ap())
                    for h_ in range(HPC):
                        nc.sync.dma_start(
                            out=wout_s[:, h_ * CO:(h_ + 1) * CO],
                            in_=wout_d.ap()[h_ * D:(h_ + 1) * D, :],
                        )

                # ---- Q^T / K^T projection ----
                # 3 M-tiles: 0 = Q(h0)|Q(h1), 1 = K(h0)|K(h1), 2 = Q(h2)|K(h2).
                # K(h2) lands on psum partitions 64-127 but must live at 0-63
                # (same base as Q(h2) for the S matmul): identity-shift via PE.
                for mt in range(3):
                    ps = psum_s.tile([128, 2 * QB], F32, tag="ps")
                    for c in range(NCC):
                        nc.tensor.matmul(
                            ps[:, 0:QB],
                            lhsT=(wqk_s[:, c * 512 + mt * 128: c * 512 + (mt + 1) * 128]),
                            rhs=(xt[c][:]),
                            start=(c == 0),
                            stop=(c == NCC - 1),
                        )
                    if mt < 2:
                        dst = qt if mt == 0 else kt
                        nc.vector.tensor_scalar(
                            out=dst[:, tb * QB:(tb + 1) * QB],
                            in0=ps[:, 0:QB],
                            scalar1=bqk_s[:, mt:mt + 1],
                            scalar2=None,
                            op0=mybir.AluOpType.add,
                        )
                    else:
                        # q2 -> qt block 1 directly
                        nc.vector.tensor_scalar(
                            out=qt[0:64, T + tb * QB: T + (tb + 1) * QB],
                            in0=ps[0:64, 0:QB],
                            scalar1=bqk_s[0:64, 2:3],
                            scalar2=None,
                            op0=mybir.AluOpType.add,
                        )
                        # k2: psum[64:128] -> sbuf (bias fused) -> PE shift to base 0
                        nc.vector.tensor_scalar(
                            out=ktmp[64:128, 0:QB],
                            in0=ps[64:128, 0:QB],
                            scalar1=bqk_s[64:128, 2:3],
                            scalar2=None,
                            op0=mybir.AluOpType.add,
                        )
                        ps2 = psum_s.tile([128, 2 * QB], F32, tag="ps")
                        nc.tensor.matmul(
                            ps2[0:64, 0:QB],
                            lhsT=(ident_s[64:128, :]),
                            rhs=(ktmp[64:128, 0:QB]),
                            start=True,
                            stop=True,
                        )
                        nc.vector.tensor_copy(
                            kt[0:64, T + tb * QB: T + (tb + 1) * QB],
                            ps2[0:64, 0:QB],
                        )

                # ---- V projection (natural layout + ones) ----
                for ts in range(QB // 128):
                    pv = psum_z.tile([128, 512], F32, tag="pz")
                    for c in range(NCC):
                        nc.tensor.matmul(
                            pv[:, 0:VWP],
                            lhsT=(xt[c][:, ts * 128:(ts + 1) * 128]),
                            rhs=(wv_s[:, c * VWP:(c + 1) * VWP]),
                            start=(c == 0),
                            stop=(c == NCC - 1),
                        )
                    kc = tb * (QB // 128) + ts
                    dst = v1[:, kc * CS:(kc + 1) * CS].rearrange(
                        "p (h x) -> p h x", x=D + 1
                    )[:, :, 0:D]
                    src = pv[:, 0:VW].rearrange("p (h d) -> p h d", d=D)
                    nc.vector.tensor_copy(dst, src)

                # ---- causal attention for query block j = tb ----
                # software-pipelined: S matmuls for item i+1 are emitted before
                # exp/mask/AV of item i, flattened across the 3 heads.
                j = tb
                nb = 2 * (j + 1)
                yts = []
                pzs = {}
                pend = None
                for h in range(HPC):
                    for m in range(nb):
                        if m == 0:
                            pzs[h] = psum_z.tile(
                                [128, 512], F32, tag="pz", name=f"pz{h}"
                            )
                        ps = psum_s.tile([128, 2 * QB], F32, tag="ps")
                        qb_, ho_ = qbase(h), hoff(h)
                        for u in range(2):
                            n = 2 * m + u
                            trim = max(0, (n - 4 * j) * KC)
                            nc.tensor.matmul(
                                ps[:, u * QB + trim:(u + 1) * QB],
                                lhsT=(kt[qb_:qb_ + 64, ho_ + n * KC: ho_ + (n + 1) * KC]),
                                rhs=(qt[qb_:qb_ + 64, ho_ + j * QB + trim: ho_ + (j + 1) * QB]),
                                start=True,
                                stop=True,
                            )
                        if pend is not None:
                            emit_eav(j, pend[0], pend[1], nb, pend[2], pzs[pend[0]], yts)
                        pend = (h, m, ps)
                emit_eav(j, pend[0], pend[1], nb, pend[2], pzs[pend[0]], yts)

                # ---- output projection for this query block ----
                for ts in range(QB // 128):
                    ot = out_pool.tile([128, CO], F32, tag="ot")
                    for half in range(2):
                        po = psum_o.tile([128, 384], F32, tag="po")
                        for h in range(HPC):
                            nc.tensor.matmul(
                                po[:],
                                lhsT=(yts[h][:, ts * 128:(ts + 1) * 128]),
                                rhs=(wout_s[:, h * CO + half * 384: h * CO + (half + 1) * 384]),
                                start=(h == 0),
                                stop=(h == HPC - 1),
                            )
                        nc.vector.tensor_copy(ot[:, half * 384:(half + 1) * 384], po[:])
                    row = j * QB + ts * 128
                    nc.sync.dma_start(out=out_d.ap()[row:row + 128, :], in_=ot[:])

    nc.compile()
    return nc


def make_masks():
    """Diagonal-block masks: masks[k, c*512 + q] = 1.0 iff 128*c + k <= q."""
    QB = 512
    m = np.zeros((128, 4 * QB), dtype=np.float32)
    for c in range(4):
        k = np.arange(128)[:, None]
        q = np.arange(QB)[None, :]
        m[:, c * QB:(c + 1) * QB] = (128 * c + k <= q).astype(np.float32)
    return m


def make_core_inputs(x, Wqkv, bqkv, core, T=T_FULL):
    """Host-side shard prep for one core."""
    b, g = divmod(core, GROUPS)
    h0 = HPC * g  # first global head of this core
    xT = np.ascontiguousarray(x[b].T).astype(np.float32)          # [C, T]

    def wq(h):
        return Wqkv[:, h * D:(h + 1) * D]

    def wk(h):
        return Wqkv[:, C + h * D: C + (h + 1) * D]

    def bq(h):
        return bqkv[h * D:(h + 1) * D]

    def bk(h):
        return bqkv[C + h * D: C + (h + 1) * D]

    z64 = np.zeros((C, D), dtype=np.float32)
    wqk = np.concatenate(
        [wq(h0), wq(h0 + 1), wk(h0), wk(h0 + 1), wq(h0 + 2), wk(h0 + 2), z64, z64],
        axis=1,
    ).astype(np.float32)                                           # [C, 512]
    zb = np.zeros(D, dtype=np.float32)
    bqk = np.stack(
        [
            np.concatenate([bq(h0), bq(h0 + 1)]),
            np.concatenate([bk(h0), bk(h0 + 1)]),
            np.concatenate([bq(h0 + 2), bk(h0 + 2)]),
            np.concatenate([zb, zb]),
        ],
        axis=1,
    ).astype(np.float32)                                           # [128, 4]
    wv = np.zeros((C, 256), dtype=np.float32)
    wv[:, : HPC * D] = Wqkv[:, 2 * C + g * HPC * D: 2 * C + (g + 1) * HPC * D]
    return {
        "xT": xT,
        "wqk": np.ascontiguousarray(wqk),
        "bqk": np.ascontiguousarray(bqk),
        "wv": wv,
        "masks": make_masks(),
        "ident": np.concatenate(
            [np.zeros((64, 64), np.float32), np.eye(64, dtype=np.float32)]
        ),
    }


_NC_CACHE = {}


def kernel(x, Wqkv, bqkv, Wout, bout):
    x = np.asarray(x, dtype=np.float32)
    Wqkv = np.asarray(Wqkv, dtype=np.float32)
    bqkv = np.asarray(bqkv, dtype=np.float32)
    Wout = np.asarray(Wout, dtype=np.float32)
    bout = np.asarray(bout, dtype=np.float32)
    T = x.shape[1]

    if T not in _NC_CACHE:
        _NC_CACHE[T] = build_nc(T)
    nc = _NC_CACHE[T]

    in_maps = []
    for core in range(N_CORES):
        b, g = divmod(core, GROUPS)
        m = make_core_inputs(x, Wqkv, bqkv, core, T)
        m["wout"] = np.ascontiguousarray(Wout[g * HPC * D:(g + 1) * HPC * D, :])
        in_maps.append(m)

    trace = bool(int(os.environ.get("KERNEL_TRACE", "0")))
    res = bass_utils.run_bass_kernel_spmd(
        nc, in_maps, core_ids=list(range(N_CORES)), trace=trace,
    )
    if trace and res.exec_time_ns is not None:
        print(f"HW exec time: {res.exec_time_ns} ns")
        if res.instructions_and_trace is not None:
            print(f"trace: {res.instructions_and_trace[1]}")

    out = np.zeros((B, T, C), dtype=np.float32)
    for b in range(B):
        for g in range(GROUPS):
            out[b] += res.results[b * GROUPS + g]["out"]
    # host bias compensation: v-bias flows through Wout as a constant row; + bout
    extra = bqkv[2 * C: 3 * C] @ Wout + bout
    out += extra[None, None, :]
    return out


# revision 8
# speedup vs baseline: 1.3666x; 1.2195x over previous
"""Trainium2 Bass kernel for causal multi-head attention (B=2, T=4096, C=768, H=12).

Sharding: 8 cores = 2 (batch) x 4 (head groups of 3). Each core computes, for its
batch element b and its 3 heads:
  - Q^T/K^T projection (transposed layout, fused q/k bias)
  - V projection (natural layout, ones column appended for free softmax sums)
  - causal flash attention in S^T = [keys, queries] orientation
  - row-sharded output projection -> partial [T, C] output
Host sums the 4 partial outputs per batch element and adds the bias terms.

All matmuls run as float32r (FP22: 1 row/cycle at N>=256), fp32 accumulate.

v2 structure (from trace analysis of the v1 baseline):
  - Q^T and K^T live in SEPARATE SBUF tiles (same-tile operands serialize the PE).
  - The flash inner loop is software-pipelined: the S matmuls of batch m+1 are
    emitted before exp/mask/AV of batch m, so the PE never waits on the ACT
    engine; batches are flattened across the 3 heads of a block.
  - Softmax denominators use a DVE reciprocal (reciprocal_approx_fast) instead
    of Ln/Exp on ACT: the Ln table load thrashed the ACT tables 48x per core.

Layout note: the PE requires lhsT and rhs at the same SBUF base partition, so
Q^T/K^T are packed as: free block 0 = h0@p0-63 | h1@p64-127, block 1 = h2@p0-63.
"""

import os
import sys

for _p in ("/opt/trn_rl_repo", "/root/.axon_site/_ro/trn_rl_repo"):
    if os.path.isdir(_p) and _p not in sys.path:
        sys.path.insert(0, _p)

import ml_dtypes
import numpy as np

import concourse.bass as bass
import concourse.mybir as mybir
import concourse.tile as tile
from concourse import bacc, bass_utils

# Problem constants (hardcoded per harness contract)
B, T_FULL, C = 2, 4096, 768
H, D = 12, 64
N_CORES = 8
HPC = 3          # heads per core
GROUPS = 4       # head groups (cores per batch element)

F32 = mybir.dt.float32
F32R = mybir.dt.float32r


def _np_dt(dt):
    return ml_dtypes.bfloat16 if dt == mybir.dt.bfloat16 else np.float32


def build_nc(T=T_FULL):
    """Build the per-core Bass module. T must be a multiple of 512."""
    QB = 512                 # query block
    KC = 128                 # key chunk
    NTB = T // QB            # token blocks
    NCC = C // 128           # contraction chunks (6)
    NKC = T // KC            # key chunks total
    CS = HPC * (D + 1)       # V|ones chunk stride (195)
    CO = C                   # output channels
    VW = HPC * D             # v width (192)
    VWP = 256                # padded v proj width

    nc = bacc.Bacc(None, target_bir_lowering=False, debug=False)

    xT_d = nc.dram_tensor("xT", [C, T], F32R, kind="ExternalInput")
    wqk_d = nc.dram_tensor("wqk", [C, 4 * 128], F32R, kind="ExternalInput")
    bqk_d = nc.dram_tensor("bqk", [128, 4], F32, kind="ExternalInput")
    wv_d = nc.dram_tensor("wv", [C, VWP], F32R, kind="ExternalInput")
    wout_d = nc.dram_tensor("wout", [VW, CO], F32R, kind="ExternalInput")
    masks_d = nc.dram_tensor("masks", [128, 4 * QB], F32R, kind="ExternalInput")
    ident_d = nc.dram_tensor("ident", [128, 64], F32R, kind="ExternalInput")
    out_d = nc.dram_tensor("out", [T, CO], F32, kind="ExternalOutput")

    # per-head addressing into qt/kt tiles: block 0 = h0@p0-63|h1@p64-127,
    # block 1 = h2@p0-63
    def qbase(h):
        return 64 if h == 1 else 0

    def hoff(h):
        return T if h == 2 else 0

    with tile.TileContext(nc) as tc:
        with (
            tc.tile_pool(name="singles", bufs=1) as singles,
            tc.tile_pool(name="xt", bufs=12) as xt_pool,
            tc.tile_pool(name="e", bufs=4) as e_pool,
            tc.tile_pool(name="yt", bufs=6) as yt_pool,
            tc.tile_pool(name="nrm", bufs=3) as nrm_pool,
            tc.tile_pool(name="ostage", bufs=3) as out_pool,
            tc.tile_pool(name="ps", bufs=3, space="PSUM") as psum_s,
            tc.tile_pool(name="pz", bufs=2, space="PSUM") as psum_z,
        ):
            # Persistent SBUF tensors
            qt = singles.tile([128, 2 * T], F32R)
            kt = singles.tile([128, 2 * T], F32R)
            v1 = singles.tile([128, NKC * CS], F32R)      # V|ones, keys on partitions
            wqk_s = singles.tile([128, NCC * 512], F32R)
            wv_s = singles.tile([128, NCC * VWP], F32R)
            wout_s = singles.tile([64, HPC * CO], F32R)
            masks_s = singles.tile([128, 4 * QB], F32R)
            bqk_s = singles.tile([128, 4], F32)
            ident_s = singles.tile([128, 64], F32R)
            ktmp = singles.tile([128, 512], F32R)

            # ones columns of v1 (memset can't write f32r; DVE copy rounds)
            ones_c = singles.tile([128, 1], F32)
            nc.vector.memset(ones_c[:], 1.0)
            ones_dst = v1[:].rearrange("p (k h x) -> p k h x", h=HPC, x=D + 1)[
                :, :, :, D:D + 1
            ]
            nc.vector.tensor_copy(ones_dst, ones_c.to_broadcast([128, NKC, HPC, 1]))
            nc.sync.dma_start(out=bqk_s[:], in_=bqk_d.ap())
            nc.sync.dma_start(out=ident_s[:], in_=ident_d.ap())

            def emit_eav(j, h, m, nb, ps, pz, yts):
                """exp -> mask -> AV for batch (h, m); normalize at last batch."""
                e = e_pool.tile([128, 2 * QB], F32R, tag="e")
                trims = [max(0, (2 * m + u - 4 * j) * KC) for u in range(2)]
                if trims[0] == 0 and trims[1] == 0:
                    nc.scalar.activation(
                        e[:], ps[:], mybir.ActivationFunctionType.Exp,
                        scale=0.125,
                    )
                else:
                    for u in range(2):
                        lo = u * QB + trims[u]
                        hi = (u + 1) * QB
                        nc.scalar.activation(
                            e[:, lo:hi], ps[:, lo:hi],
                            mybir.ActivationFunctionType.Exp,
                            scale=0.125,
                        )
                for u in range(2):
                    n = 2 * m + u
                    cdiag = n - 4 * j
                    if cdiag >= 0:
                        trim = cdiag * KC
                        nc.vector.tensor_mul(
                            e[:, u * QB + trim:(u + 1) * QB],
                            e[:, u * QB + trim:(u + 1) * QB],
                            masks_s[:, cdiag * QB + trim:(cdiag + 1) * QB],
                        )
                for u in range(2):
                    n = 2 * m + u
                    trim = max(0, (n - 4 * j) * KC)
                    nc.tensor.matmul(
                        pz[0:D + 1, trim:QB],
                        lhsT=(v1[:, n * CS + h * (D + 1): n * CS + (h + 1) * (D + 1)]),
                        rhs=(e[:, u * QB + trim:(u + 1) * QB]),
                        start=(m == 0 and u == 0),
                        stop=(m == nb - 1 and u == 1),
                    )
                if m == nb - 1:
                    # normalize: y = z * (1/sums). ACT Copy stages the sums row
                    # from psum partition 64 to sbuf partition 0 (Copy is in
                    # every ACT table set -> no table reload, unlike the v1
                    # Ln/Exp trick which thrashed tables 48x). Reciprocal runs
                    # on DVE; broadcast along partitions on GPSIMD.
                    sums = nrm_pool.tile([1, QB], F32, tag="sums")
                    nc.scalar.activation(
                        sums[:], pz[D:D + 1, 0:QB],
                        mybir.ActivationFunctionType.Copy,
                    )
                    rc = nrm_pool.tile([1, QB], F32, tag="rc")
                    nc.vector.reciprocal_approx_fast(out=rc[:], in_=sums[:])
                    bc = nrm_pool.tile([64, QB], F32, tag="bc")
                    nc.gpsimd.partition_broadcast(bc[:], rc[:])
                    yt = yt_pool.tile([64, QB], F32R, tag="yt")
                    nc.vector.tensor_mul(yt[:], pz[0:D, 0:QB], bc[:])
                    yts.append(yt)

            def issue_xt_dma(tb):
                lst = []
                for c in range(NCC):
                    t_ = xt_pool.tile([128, QB], F32R, tag="xt", name=f"xt{tb}_{c}")
                    nc.sync.dma_start(
                        out=t_[:],
                        in_=xT_d.ap()[c * 128:(c + 1) * 128, tb * QB:(tb + 1) * QB],
                    )
                    lst.append(t_)
                return lst

            xt_next = None
            for tb in range(NTB):
                # ---- x^T tiles: block 0 loads now, later blocks were
                # prefetched one block ahead ----
                if tb == 0:
                    xt = issue_xt_dma(0)
                    for c in range(NCC):
                        nc.sync.dma_start(
                            out=wqk_s[:, c * 512:(c + 1) * 512],
                            in_=wqk_d.ap()[c * 128:(c + 1) * 128, :],
                        )
                else:
                    xt = xt_next
                if tb + 1 < NTB:
                    xt_next = issue_xt_dma(tb + 1)
                if tb == 0:
                    for c in range(NCC):
                        nc.sync.dma_start(
                            out=wv_s[:, c * VWP:(c + 1) * VWP],
                            in_=wv_d.ap()[c * 128:(c + 1) * 128, :],
                        )
                    nc.sync.dma_start(out=masks_s[:], in_=masks_d.ap())
                    for h_ in range(HPC):
                        nc.sync.dma_start(
                            out=wout_s[:, h_ * CO:(h_ + 1) * CO],
                            in_=wout_d.ap()[h_ * D:(h_ + 1) * D, :],
                        )

                # ---- Q^T / K^T projection ----
                # 3 M-tiles: 0 = Q(h0)|Q(h1), 1 = K(h0)|K(h1), 2 = Q(h2)|K(h2).
                # K(h2) lands on psum partitions 64-127 but must live at 0-63
                # (same base as Q(h2) for the S matmul): identity-shift via PE.
                for mt in range(3):
                    ps = psum_s.tile([128, 2 * QB], F32, tag="ps")
                    for c in range(NCC):
                        nc.tensor.matmul(
                            ps[:, 0:QB],
                            lhsT=(wqk_s[:, c * 512 + mt * 128: c * 512 + (mt + 1) * 128]),
                            rhs=(xt[c][:]),
                            start=(c == 0),
                            stop=(c == NCC - 1),
                        )
                    if mt < 2:
                        dst = qt if mt == 0 else kt
                        nc.vector.tensor_scalar(
                            out=dst[:, tb * QB:(tb + 1) * QB],
                            in0=ps[:, 0:QB],
                            scalar1=bqk_s[:, mt:mt + 1],
                            scalar2=None,
                            op0=mybir.AluOpType.add,
                        )
                    else:
                        # q2 -> qt block 1 directly
                        nc.vector.tensor_scalar(
                            out=qt[0:64, T + tb * QB: T + (tb + 1) * QB],
                            in0=ps[0:64, 0:QB],
                            scalar1=bqk_s[0:64, 2:3],
                            scalar2=None,
                            op0=mybir.AluOpType.add,
                        )
                        # k2: psum[64:128] -> sbuf (bias fused) -> PE shift to base 0
                        nc.vector.tensor_scalar(
                            out=ktmp[64:128, 0:QB],
                            in0=ps[64:128, 0:QB],
                            scalar1=bqk_s[64:128, 2:3],
                            scalar2=None,
                            op0=mybir.AluOpType.add,
                        )
                        ps2 = psum_s.tile([128, 2 * QB], F32, tag="ps")
                        nc.tensor.matmul(
                            ps2[0:64, 0:QB],
                            lhsT=(ident_s[64:128, :]),
                            rhs=(ktmp[64:128, 0:QB]),
                            start=True,
                            stop=True,
                        )
                        nc.vector.tensor_copy(
                            kt[0:64, T + tb * QB: T + (tb + 1) * QB],
                            ps2[0:64, 0:QB],
                        )

                # ---- V projection (natural layout + ones) ----
                for ts in range(QB // 128):
                    pv = psum_z.tile([128, 512], F32, tag="pz")
                    for c in range(NCC):
                        nc.tensor.matmul(
                            pv[:, 0:VWP],
                            lhsT=(xt[c][:, ts * 128:(ts + 1) * 128]),
                            rhs=(wv_s[:, c * VWP:(c + 1) * VWP]),
                            start=(c == 0),
                            stop=(c == NCC - 1),
                        )
                    kc = tb * (QB // 128) + ts
                    dst = v1[:, kc * CS:(kc + 1) * CS].rearrange(
                        "p (h x) -> p h x", x=D + 1
                    )[:, :, 0:D]
                    src = pv[:, 0:VW].rearrange("p (h d) -> p h d", d=D)
                    nc.vector.tensor_copy(dst, src)

                # ---- causal attention for query block j = tb ----
                # software-pipelined: S matmuls for item i+1 are emitted before
                # exp/mask/AV of item i, flattened across the 3 heads.
                j = tb
                nb = 2 * (j + 1)
                yts = []
                pzs = {}
                pending = []
                for h in range(HPC):
                    for m in range(nb):
                        if m == 0:
                            pzs[h] = psum_z.tile(
                                [128, 512], F32, tag="pz", name=f"pz{h}"
                            )
                        ps = psum_s.tile([128, 2 * QB], F32, tag="ps")
                        qb_, ho_ = qbase(h), hoff(h)
                        for u in range(2):
                            n = 2 * m + u
                            trim = max(0, (n - 4 * j) * KC)
                            nc.tensor.matmul(
                                ps[:, u * QB + trim:(u + 1) * QB],
                                lhsT=(kt[qb_:qb_ + 64, ho_ + n * KC: ho_ + (n + 1) * KC]),
                                rhs=(qt[qb_:qb_ + 64, ho_ + j * QB + trim: ho_ + (j + 1) * QB]),
                                start=True,
                                stop=True,
                            )
                        pending.append((h, m, ps))
                        if len(pending) > 2:
                            ph, pm, pps = pending.pop(0)
                            emit_eav(j, ph, pm, nb, pps, pzs[ph], yts)
                for ph, pm, pps in pending:
                    emit_eav(j, ph, pm, nb, pps, pzs[ph], yts)
                pending = []

                # ---- output projection for this query block ----
                for ts in range(QB // 128):
                    ot = out_pool.tile([128, CO], F32, tag="ot")
                    for half in range(2):
                        po = psum_z.tile([128, 512], F32, tag="pz", name="po")
                        for h in range(HPC):
                            nc.tensor.matmul(
                                po[:, 0:384],
                                lhsT=(yts[h][:, ts * 128:(ts + 1) * 128]),
                                rhs=(wout_s[:, h * CO + half * 384: h * CO + (half + 1) * 384]),
                                start=(h == 0),
                                stop=(h == HPC - 1),
                            )
                        nc.vector.tensor_copy(
                            ot[:, half * 384:(half + 1) * 384], po[:, 0:384]
                        )
                    row = j * QB + ts * 128
                    nc.sync.dma_start(out=out_d.ap()[row:row + 128, :], in_=ot[:])

    nc.compile()
    return nc


def make_masks():
    """Diagonal-block masks: masks[k, c*512 + q] = 1.0 iff 128*c + k <= q."""
    QB = 512
    m = np.zeros((128, 4 * QB), dtype=np.float32)
    for c in range(4):
        k = np.arange(128)[:, None]
        q = np.arange(QB)[None, :]
        m[:, c * QB:(c + 1) * QB] = (128 * c + k <= q).astype(np.float32)
    return m


def make_core_inputs(x, Wqkv, bqkv, core, T=T_FULL):
    """Host-side shard prep for one core."""
    b, g = divmod(core, GROUPS)
    h0 = HPC * g  # first global head of this core
    xT = np.ascontiguousarray(x[b].T).astype(np.float32)          # [C, T]

    def wq(h):
        return Wqkv[:, h * D:(h + 1) * D]

    def wk(h):
        return Wqkv[:, C + h * D: C + (h + 1) * D]

    def bq(h):
        return bqkv[h * D:(h + 1) * D]

    def bk(h):
        return bqkv[C + h * D: C + (h + 1) * D]

    z64 = np.zeros((C, D), dtype=np.float32)
    wqk = np.concatenate(
        [wq(h0), wq(h0 + 1), wk(h0), wk(h0 + 1), wq(h0 + 2), wk(h0 + 2), z64, z64],
        axis=1,
    ).astype(np.float32)                                           # [C, 512]
    zb = np.zeros(D, dtype=np.float32)
    bqk = np.stack(
        [
            np.concatenate([bq(h0), bq(h0 + 1)]),
            np.concatenate([bk(h0), bk(h0 + 1)]),
            np.concatenate([bq(h0 + 2), bk(h0 + 2)]),
            np.concatenate([zb, zb]),
        ],
        axis=1,
    ).astype(np.float32)                                           # [128, 4]
    wv = np.zeros((C, 256), dtype=np.float32)
    wv[:, : HPC * D] = Wqkv[:, 2 * C + g * HPC * D: 2 * C + (g + 1) * HPC * D]
    return {
        "xT": xT,
        "wqk": np.ascontiguousarray(wqk),
        "bqk": np.ascontiguousarray(bqk),
        "wv": wv,
        "masks": make_masks(),
        "ident": np.concatenate(
            [np.zeros((64, 64), np.float32), np.eye(64, dtype=np.float32)]
        ),
    }


_NC_CACHE = {}


def kernel(x, Wqkv, bqkv, Wout, bout):
    x = np.asarray(x, dtype=np.float32)
    Wqkv = np.asarray(Wqkv, dtype=np.float32)
    bqkv = np.asarray(bqkv, dtype=np.float32)
    Wout = np.asarray(Wout, dtype=np.float32)
    bout = np.asarray(bout, dtype=np.float32)
    T = x.shape[1]

    if T not in _NC_CACHE:
        _NC_CACHE[T] = build_nc(T)
    nc = _NC_CACHE[T]

    in_maps = []
    for core in range(N_CORES):
        b, g = divmod(core, GROUPS)
        m = make_core_inputs(x, Wqkv, bqkv, core, T)
        m["wout"] = np.ascontiguousarray(Wout[g * HPC * D:(g + 1) * HPC * D, :])
        in_maps.append(m)

    trace = bool(int(os.environ.get("KERNEL_TRACE", "0")))
    res = bass_utils.run_bass_kernel_spmd(
        nc, in_maps, core_ids=list(range(N_CORES)), trace=trace,
    )
    if trace and res.exec_time_ns is not None:
        print(f"HW exec time: {res.exec_time_ns} ns")
        if res.instructions_and_trace is not None:
            print(f"trace: {res.instructions_and_trace[1]}")

    out = np.zeros((B, T, C), dtype=np.float32)
    for b in range(B):
        for g in range(GROUPS):
            out[b] += res.results[b * GROUPS + g]["out"]
    # host bias compensation: v-bias flows through Wout as a constant row; + bout
    extra = bqkv[2 * C: 3 * C] @ Wout + bout
    out += extra[None, None, :]
    return out
